# revision 3
# baseline (speedup 1.0000x reference)
"""Fused dequant-GEMM (quint8 affine) on 8 TRN2 NeuronCores.

out = ((x - 65) * 0.199) @ ((y - 160) * 0.0215),  x,y: [4096, 4096] uint8-valued int32.

Strategy (tensor-parallel, per sharding hint):
  - Shard y column-wise into 8 blocks of 512; replicate x. No collectives.
  - Host pre-packs both operands as zero-point-shifted bf16 (integers in
    [-160, 190] are exact in bf16), laid out so DMA lines are contiguous per
    partition and SBUF tiles are directly consumable as matmul operands
    (K on the partition axis).
  - Each core processes m-tiles in groups of 4 with a k-major inner order:
    the 4 PSUM banks accumulate in parallel, so the y stream (8 chunked DMAs
    on the gpsimd queue) is consumed at 1/4 the rate it arrives — no PE
    stall while y streams in during the first group. x chunks stream on the
    sync queue. A throwaway matmul burst at t~3.6us warms the PE HAM clock
    gate before real data lands.
  - Epilogue per m-tile: ScalarE multiplies PSUM by 0.199*0.0215 on the way
    to SBUF fp32, then DMA out. Host concatenates the 8 [4096, 512] outputs.
"""

import numpy as np
import ml_dtypes

M = 4096
K = 4096
N = 4096
NCORES = 8
P = 128
NSH = N // NCORES  # 512 columns per core
MT = M // P        # 32 m-tiles
KT = K // P        # 32 k-tiles

YC = 4             # k-tiles per y chunk
NYC = KT // YC     # 8 y chunks
XCK = 8            # k-tiles per x chunk
NXC = KT // XCK    # 4 x chunks per m-tile
G = 4              # m-tiles per group (PSUM banks used per group)
NG = MT // G       # 8 groups
N_WARMUP_MM = 12   # throwaway matmuls to trip the HAM clock gate

ZP_X = 65.0
ZP_Y = 160.0
# Match the reference's fp32 scale arithmetic as closely as possible.
SCALE = float(np.float32(0.199) * np.float32(0.0215))

_CACHE = {}


def build_nc():
    """Build + compile the per-core Bass graph (identical on all 8 cores)."""
    from concourse import bass, bacc, tile, mybir

    nc = bacc.Bacc("TRN2", target_bir_lowering=False, debug=False)
    bf16 = mybir.dt.bfloat16
    f32 = mybir.dt.float32

    # x packed as [mt, p=k%128, kt*128+m] -> contiguous per partition row
    x_d = nc.dram_tensor("x", [MT, P, K], bf16, kind="ExternalInput").ap()
    # y shard packed as [p=k%128, kt*512+n] -> 32KB contiguous per partition
    y_d = nc.dram_tensor("y", [P, KT * NSH], bf16, kind="ExternalInput").ap()
    # out as [mt, m, n]
    o_d = nc.dram_tensor("out", [MT, P, NSH], f32, kind="ExternalOutput").ap()

    with tile.TileContext(nc) as tc:
        with (
            tc.tile_pool(name="wpool", bufs=1) as wpool,
            tc.tile_pool(name="ypool", bufs=NYC) as ypool,
            tc.tile_pool(name="xpool", bufs=4 * G * NXC) as xpool,
            tc.tile_pool(name="opool", bufs=2 * G) as opool,
            tc.tile_pool(name="ppool", bufs=8, space=bass.MemorySpace.PSUM) as ppool,
        ):
            # PE warm-up: zeroed operands, results never read. Keeps the PE
            # busy during the initial DMA window so HAM unthrottles to 2.4GHz
            # before the first real matmul.
            wx = wpool.tile([P, P], bf16)
            wr = wpool.tile([P, NSH], bf16)
            nc.vector.memset(wx[:], 0.0)
            nc.vector.memset(wr[:], 0.0)
            wps = ppool.tile([P, NSH], f32, name="wps", tag="ps")
            for _ in range(N_WARMUP_MM):
                nc.tensor.matmul(wps[:], wx[:], wr[:], start=True, stop=True)

            # y: 8 chunks on the gpsimd DMA queue (parallel to x on sync)
            y_ts = []
            for c in range(NYC):
                y_t = ypool.tile([P, YC * NSH], bf16, name="y_t", tag="y_t")
                nc.gpsimd.dma_start(
                    y_t[:], y_d[:, c * YC * NSH:(c + 1) * YC * NSH]
                )
                y_ts.append(y_t)

            for g in range(NG):
                # x chunk DMAs: chunk-major so every m-tile's first chunk
                # lands before any of their second chunks.
                xcs = [[None] * NXC for _ in range(G)]
                for c in range(NXC):
                    for m in range(G):
                        mt = g * G + m
                        t = xpool.tile([P, XCK * P], bf16, name="xc", tag="xc")
                        nc.sync.dma_start(
                            t[:], x_d[mt][:, c * XCK * P:(c + 1) * XCK * P]
                        )
                        xcs[m][c] = t

                ps = [ppool.tile([P, NSH], f32, name="ps", tag="ps") for _ in range(G)]
                for kt in range(KT):
                    for m in range(G):
                        nc.tensor.matmul(
                            ps[m][:],
                            xcs[m][kt // XCK][:, (kt % XCK) * P:(kt % XCK + 1) * P],
                            y_ts[kt // YC][:, (kt % YC) * NSH:(kt % YC + 1) * NSH],
                            start=(kt == 0),
                            stop=(kt == KT - 1),
                        )

                for m in range(G):
                    mt = g * G + m
                    o_t = opool.tile([P, NSH], f32, name="o_t", tag="o_t")
                    nc.scalar.mul(o_t[:], ps[m][:], SCALE)
                    nc.sync.dma_start(o_d[mt], o_t[:])

    nc.compile()
    return nc


def prep_in_maps(x, y):
    """Shift zero-points, cast to bf16 (exact for these integer ranges), and
    pack for partition-contiguous DMA. Returns one in_map per core."""
    bf16 = ml_dtypes.bfloat16
    x = np.asarray(x)
    y = np.asarray(y)

    xd = (x.astype(np.float32) - np.float32(ZP_X)).astype(bf16)  # [M, K]
    # [mt, m, kt, p] -> [mt, p, kt, m]
    xp = np.ascontiguousarray(
        xd.reshape(MT, P, KT, P).transpose(0, 3, 2, 1)
    ).reshape(MT, P, K)

    yd = (y.astype(np.float32) - np.float32(ZP_Y)).astype(bf16)  # [K, N]
    # [kt, p, n] -> [p, kt, n]
    yp = yd.reshape(KT, P, N).transpose(1, 0, 2)

    in_maps = []
    for c in range(NCORES):
        ysh = np.ascontiguousarray(yp[:, :, c * NSH:(c + 1) * NSH]).reshape(
            P, KT * NSH
        )
        in_maps.append({"x": xp, "y": ysh})
    return in_maps


def assemble_output(results):
    cols = [np.asarray(r["out"], dtype=np.float32).reshape(M, NSH) for r in results]
    return np.concatenate(cols, axis=1)


def get_nc():
    if "nc" not in _CACHE:
        _CACHE["nc"] = build_nc()
    return _CACHE["nc"]


def kernel(x, y):
    from concourse.bass_utils import run_bass_kernel_spmd

    nc = get_nc()
    in_maps = prep_in_maps(x, y)
    res = run_bass_kernel_spmd(nc, in_maps, core_ids=list(range(NCORES)))
    out = assemble_output(res.results)
    if np.isnan(out).any():
        # Cold-start insurance: a fresh device stack once produced NaN on the
        # very first execution; a retry has always been clean.
        res = run_bass_kernel_spmd(nc, in_maps, core_ids=list(range(NCORES)))
        out = assemble_output(res.results)
    return out
